# revision 40
# baseline (speedup 1.0000x reference)
"""Multi-head attention (B=4, S=2048, D=1024, H=16) on 8 trn2 NeuronCores.

Sharding: core c handles batch c//2 and heads (c%2)*8 .. (c%2)*8+8.
Each core computes its partial output through the fc projection; the host
sums the two per-batch partials.

Device dataflow (per core), everything fp16 inputs / fp32 accumulate:
  1. Project k, q into head-transposed layout  khT/qhT [c, token]
  2. Project v into  vhc [token, c]  with an appended ones column
  3. Per (head-pair, q-block): scores^T = kh^T q  [k, q] in PSUM (the two
     heads run concurrently in the PE array via row tiling), exp via ACT
     with a per-key bias (-50 for masked/padded keys, folding the
     key-padding mask), then P~^T + denominator via a [V | 1] matmul.
  4. Softmax denominators travel through DRAM, get reciprocated on a
     128-lane layout, broadcast back via partition-stride-0 DMA reads,
     and multiplied into P~^T -> ctxT.
  5. fc projection from ctxT -> partial output.

Keys are compacted on the host: masked keys (mask==1) are dropped and the
remainder zero-padded to SK=1152, cutting attention work ~44%.  The -50
exp-bias makes padded keys contribute exp(-50), which underflows to an
exact 0 in fp16.  All matmul operands are fp16 (host-cast; 10-bit
mantissa) with fp32 PSUM accumulation; softmax denominators and the
normalize stay fp32.  End-to-end scale-relative error ~6e-4.

Projection / fc work is interleaved into the attention loops as "filler"
so the PE array activity stays high (keeps the HAM clock gate at 2.4 GHz)
and no engine sits idle between phases.
"""

import numpy as np

import concourse.bass as bass
import concourse.tile as tile
from concourse import mybir
from concourse.bass_utils import run_bass_kernel_spmd

B, S, DM = 4, 2048, 1024
NH, DEPTH = 16, 64
NCORES = 8
HPC = 8                 # heads per core
C = HPC * DEPTH         # 512 output channels per core
SK = 1152               # compacted+padded key count
KC = SK // 128          # 9 key chunks
QW = 1024               # q-block width (bf16 moving limit)
NQW = S // QW           # 2
DC = DM // 128          # 8 contraction chunks
NPAIR = HPC // 2        # 4 head pairs (= c-tiles of 128)
SCALE = 1.0 / 8.0       # 1/sqrt(depth)
MASK_BIAS = -50.0

F32 = mybir.dt.float32
F32R = mybir.dt.float32r
BF16 = mybir.dt.bfloat16
FP16 = mybir.dt.float16
EXP = mybir.ActivationFunctionType.Exp


def _split_excess_waits(nc, cap_default=1, cap_evsem=2):
    """walrus in this env rejects >1 sync wait per instruction (2 for event
    semaphores); hoist excess waits onto preceding same-engine NoOps."""
    n_split = 0
    for f in nc.m.functions:
        for bb in f.blocks:
            insts = list(bb.instructions)
            out = []
            for inst in insts:
                si = inst.sync_info
                cap = cap_evsem if isinstance(inst, mybir.InstEventSemaphore) else cap_default
                if si is not None and si.on_wait and len(si.on_wait) > cap:
                    waits = list(si.on_wait)
                    extra, keep = waits[:-cap], waits[-cap:]
                    for i, w in enumerate(extra):
                        nop = mybir.InstNoOp(
                            name=f"{inst.name}_waitsplit_{i}",
                            sync_info=mybir.SyncInfo(on_wait=[w], on_update=[]),
                            bass_nofuse=True,
                            engine=inst.engine,
                        )
                        nc.register_instruction(nop, overwrite=True)
                        out.append(nop)
                    inst.sync_info = mybir.SyncInfo(on_wait=keep, on_update=list(si.on_update))
                    n_split += 1
                out.append(inst)
            if n_split:
                bb.instructions = out
    return n_split


def _emit(tc, t):
    nc = tc.nc
    from contextlib import ExitStack
    ctx = ExitStack()

    persist = ctx.enter_context(tc.tile_pool(name="persist", bufs=1))
    p_xrv = ctx.enter_context(tc.tile_pool(name="xrv", bufs=2))
    p_a = ctx.enter_context(tc.tile_pool(name="apool", bufs=8))
    p_dinvb = ctx.enter_context(tc.tile_pool(name="dinvb", bufs=2))
    p_small = ctx.enter_context(tc.tile_pool(name="small", bufs=2))
    p_fcr = ctx.enter_context(tc.tile_pool(name="fcr", bufs=8))
    p_out = ctx.enter_context(tc.tile_pool(name="outsb", bufs=2))
    p_s = ctx.enter_context(tc.tile_pool(name="pss", bufs=4, space="PSUM"))
    p_pv = ctx.enter_context(tc.tile_pool(name="pspv", bufs=3, space="PSUM"))
    p_pj = ctx.enter_context(tc.tile_pool(name="pspj", bufs=1, space="PSUM"))

    # persistent buffers
    wq_r = persist.tile([128, DC, C], FP16, tag="wq")
    wk_r = persist.tile([128, DC, C], FP16, tag="wk")
    wv_r = persist.tile([128, DC, C], FP16, tag="wv")
    xq_r = persist.tile([128, DC, S], FP16, tag="xq")
    xk_r = persist.tile([128, DC, SK], FP16, tag="xk")
    qhT = persist.tile([128, NPAIR, S], FP16, tag="qhT")
    khT = persist.tile([128, NPAIR, SK], FP16, tag="khT")
    vhc = persist.tile([128, KC, HPC, DEPTH + 1], FP16, tag="vhc")
    ctxT = persist.tile([128, NPAIR, S], FP16, tag="ctxT")
    maskb = persist.tile([128, KC], F32, tag="maskb")
    ones1 = persist.tile([128, 1], F32, tag="ones1")

    # internal DRAM for the denominator shuttle: row r=pair*32+hh*16+qq of 128
    d_dram = nc.dram_tensor("d_dram", (NPAIR * 32, 128), F32, kind="Internal").ap()
    dinv_dram = nc.dram_tensor("dinv_dram", (NPAIR * 32, 128), F32, kind="Internal").ap()
    dinv_flat = dinv_dram.rearrange("a b -> (a b)")
    # [pair, hh, qw, 1024] view for the per-(pair,qb) D-row writes
    d_view4 = d_dram.rearrange("(pr h q j) f -> pr h q (j f)", h=2, q=NQW, j=QW // 128)

    nc.sync.dma_start(maskb[:], t["maskb"])
    nc.vector.memset(ones1[:], 1.0)
    nc.vector.tensor_copy(
        vhc[:, :, :, DEPTH:DEPTH + 1],
        ones1[:].to_broadcast([128, KC, HPC, 1]),
    )

    # ---- weight + input loads (already fp16 from the host), spread
    # across four engines' DMA queues so the startup isn't issue-bound ----
    engs = [nc.sync, nc.scalar, nc.gpsimd]
    ei = 0
    for src, dst in ((t["wkT"], wk_r), (t["wqT"], wq_r), (t["wvT"], wv_r)):
        view = src.rearrange("(dc p) c -> p dc c", p=128)
        for dc in range(0, DC, 4):
            engs[ei % 3].dma_start(dst[:, dc:dc + 4, :], view[:, dc:dc + 4, :])
            ei += 1
    for src, dst, slen in ((t["kcT"], xk_r, SK), (t["qT"], xq_r, S)):
        view = src.rearrange("(dc p) s -> p dc s", p=128)
        for dc in range(DC):
            engs[ei % 3].dma_start(dst[:, dc, :], view[:, dc, :])
            ei += 1

    def proj_qk_tasks(pair):
        """Closures, each projecting one 512-token block of c-tile `pair`."""
        tasks = []
        for x_r, w_r, dst, slen in ((xk_r, wk_r, khT, SK), (xq_r, wq_r, qhT, S)):
            for tb0 in range(0, slen, 512):
                tlen = min(512, slen - tb0)

                def task(x_r=x_r, w_r=w_r, dst=dst, tb0=tb0, tlen=tlen, pair=pair):
                    ps = p_pj.tile([128, 512], F32, tag="pj",
                                   name=f"pj_{pair}_{dst.name}_{tb0}")
                    for dc in range(DC):
                        nc.tensor.matmul(ps[:, :tlen],
                                         w_r[:, dc, pair * 128:(pair + 1) * 128],
                                         x_r[:, dc, tb0:tb0 + tlen],
                                         start=(dc == 0), stop=(dc == DC - 1))
                    nc.vector.tensor_copy(dst[:, pair, tb0:tb0 + tlen], ps[:, :tlen])
                tasks.append(task)
        return tasks

    def proj_v():
        vview = t["vcT"].rearrange("(dc p) s -> p dc s", p=128)
        for kt in range(KC):
            xrv = p_xrv.tile([128, DC, 128], FP16, tag="xrv", name=f"xrv_{kt}")
            nc.sync.dma_start(xrv[:], vview[:, :, kt * 128:(kt + 1) * 128])
            ps = p_pj.tile([128, 512], F32, tag="pj", name=f"psv_{kt}")
            for dc in range(DC):
                nc.tensor.matmul(ps[:, :C], xrv[:, dc, :], wv_r[:, dc, :],
                                 start=(dc == 0), stop=(dc == DC - 1))
            nc.vector.tensor_copy(
                vhc[:, kt, :, 0:DEPTH],
                ps[:, :C].rearrange("p (h d) -> p h d", h=HPC),
            )

    def attention(pair, fillers, qw_order=None):
        """fillers: {qw: closures} consumed evenly across that qw's steps."""
        steps = 2 * KC
        for qw in (qw_order or range(NQW)):
            filler = fillers.get(qw, [])
            n_fill = len(filler)
            step = 0
            dst_stage = p_dinvb.tile([64, QW], F32, tag="dinvb",
                                     name=f"dstage_{pair}_{qw}")
            for sh in range(2):
                q0 = qw * QW + sh * 512
                pv = [p_pv.tile([DEPTH + 1, 512], F32, tag="pv",
                                name=f"pv_{pair}_{qw}_{sh}_{hh}") for hh in range(2)]
                a_prev = None
                for kc in range(KC):
                    if (n_fill and
                            step * n_fill // steps != (step + 1) * n_fill // steps):
                        filler[step * n_fill // steps]()
                    step += 1
                    cur = []
                    for hh in range(2):
                        lo = 64 * hh
                        ps_s = p_s.tile([128, 512], F32, tag="s",
                                        name=f"s_{pair}_{qw}_{sh}_{kc}_{hh}")
                        nc.tensor.matmul(ps_s[:], khT[lo:lo + 64, pair, kc * 128:(kc + 1) * 128],
                                         qhT[lo:lo + 64, pair, q0:q0 + 512],
                                         start=True, stop=True)
                        a_t = p_a.tile([128, 512], FP16, tag="A",
                                       name=f"A_{pair}_{qw}_{sh}_{kc}_{hh}")
                        nc.scalar.activation(a_t[:], ps_s[:], EXP,
                                             bias=maskb[:, kc:kc + 1], scale=SCALE)
                        cur.append(a_t)
                    if kc >= 1:
                        for hh in range(2):
                            nc.tensor.matmul(pv[hh][:], vhc[:, kc - 1, 2 * pair + hh, :],
                                             a_prev[hh][:], start=(kc == 1), stop=False)
                    a_prev = cur
                for hh in range(2):
                    nc.tensor.matmul(pv[hh][:], vhc[:, KC - 1, 2 * pair + hh, :],
                                     a_prev[hh][:], start=False, stop=True)
                for hh in range(2):
                    nc.vector.tensor_copy(dst_stage[32 * hh:32 * hh + 1, sh * 512:(sh + 1) * 512],
                                          pv[hh][DEPTH:DEPTH + 1, :])
                    nc.vector.tensor_copy(ctxT[64 * hh:64 * hh + 64, pair, q0:q0 + 512],
                                          pv[hh][0:DEPTH, :])
            for hh in range(2):
                nc.gpsimd.dma_start(d_view4[pair, hh:hh + 1, qw, :],
                                    dst_stage[32 * hh:32 * hh + 1, :])
            # denominator pipeline for this (pair, qw)
            d128 = p_small.tile([16, 128], F32, tag="d128", name=f"d128_{pair}_{qw}")
            for hh in range(2):
                r0 = pair * 32 + hh * 16 + qw * 8
                nc.sync.dma_start(d128[8 * hh:8 * hh + 8, :], d_dram[r0:r0 + 8, :])
            dinv = p_small.tile([16, 128], F32, tag="dinv", name=f"dinv_{pair}_{qw}")
            nc.vector.reciprocal(dinv[:], d128[:])
            for hh in range(2):
                r0 = pair * 32 + hh * 16 + qw * 8
                nc.sync.dma_start(dinv_dram[r0:r0 + 8, :], dinv[8 * hh:8 * hh + 8, :])
            db = p_dinvb.tile([128, QW], F32, tag="dinvb", name=f"db_{pair}_{qw}")
            for cc in range(2):
                off = pair * 4096 + cc * 2048 + qw * QW
                nc.gpsimd.dma_start(db[64 * cc:64 * cc + 64, :],
                                    dinv_flat[off:off + QW].partition_broadcast(64))
            for hh in range(2):
                sl = ctxT[64 * hh:64 * hh + 64, pair, qw * QW:(qw + 1) * QW]
                nc.vector.tensor_mul(sl, sl, db[64 * hh:64 * hh + 64, :])

    # ---- fc task construction (emission deferred) ----
    fc_view = t["fcT"].rearrange("(pr p) e -> p pr e", p=128)
    o_view = t["o"].rearrange("(tt p) e -> p tt e", p=128)
    fcrs = []
    for ec in range(2):
        for pair in range(NPAIR):
            fcr = p_fcr.tile([128, 512], FP16, tag="fcr", name=f"fcr_{ec}_{pair}")
            nc.sync.dma_start(fcr[:], fc_view[:, pair, ec * 512:(ec + 1) * 512])
            fcrs.append(fcr)

    def fc_task(tt, ec):
        def task():
            ps = p_pj.tile([128, 512], F32, tag="pj", name=f"fcps_{tt}_{ec}")
            for pair in range(NPAIR):
                nc.tensor.matmul(ps[:], ctxT[:, pair, tt * 128:(tt + 1) * 128],
                                 fcrs[ec * NPAIR + pair][:, :],
                                 start=(pair == 0), stop=(pair == NPAIR - 1))
            ob = p_out.tile([128, 512], F32, tag="outsb", name=f"ob_{tt}_{ec}")
            nc.vector.tensor_copy(ob[:], ps[:])
            nc.sync.dma_start(o_view[:, tt, ec * 512:(ec + 1) * 512], ob[:])
        return task

    # attention(3) runs qw=1 first; the fc groups that depend on qw=1
    # (tt 8..15) then fill its qw=0 half, and tt 0..7 run at the end.
    fc_fill = [fc_task(tt, ec) for tt in range(8, S // 128) for ec in range(2)]
    fc_tasks = [fc_task(tt, ec) for tt in range(8) for ec in range(2)]

    # ---- schedule: v-proj and pair-0 k/q proj up front, then per-pair
    # attention with the next pair's projections interleaved ----
    for task in proj_qk_tasks(0):
        task()
    proj_v()
    for pair in range(NPAIR):
        if pair + 1 < NPAIR:
            nxt = proj_qk_tasks(pair + 1)
            attention(pair, {0: nxt[:4], 1: nxt[4:]})
        else:
            attention(pair, {0: fc_fill}, qw_order=[1, 0])

    # ---- fc projection: 32 closures; first half fills attention(3) ----
    for task in fc_tasks:
        task()

    ctx.close()


_NC_CACHE = {}


def _get_nc():
    if "nc" in _NC_CACHE:
        return _NC_CACHE["nc"]
    nc = bass.Bass("TRN2", target_bir_lowering=False, debug=False)
    t = {
        "qT": nc.dram_tensor("qT", (DM, S), FP16, kind="ExternalInput").ap(),
        "kcT": nc.dram_tensor("kcT", (DM, SK), FP16, kind="ExternalInput").ap(),
        "vcT": nc.dram_tensor("vcT", (DM, SK), FP16, kind="ExternalInput").ap(),
        "wqT": nc.dram_tensor("wqT", (DM, C), FP16, kind="ExternalInput").ap(),
        "wkT": nc.dram_tensor("wkT", (DM, C), FP16, kind="ExternalInput").ap(),
        "wvT": nc.dram_tensor("wvT", (DM, C), FP16, kind="ExternalInput").ap(),
        "fcT": nc.dram_tensor("fcT", (C, DM), FP16, kind="ExternalInput").ap(),
        "maskb": nc.dram_tensor("maskb", (128, KC), F32, kind="ExternalInput").ap(),
        "o": nc.dram_tensor("o", (S, DM), F32, kind="ExternalOutput").ap(),
    }
    with tile.TileContext(nc) as tc:
        _emit(tc, t)
    _split_excess_waits(nc)
    _NC_CACHE["nc"] = nc
    return nc


def _in_map_for_core(core, v, k, q, mask, wq, wk, wv, fc):
    b = core // 2
    hs = (core % 2) * HPC
    sel = np.nonzero(mask[b] == 0)[0]
    n = len(sel)
    assert n <= SK, f"unmasked key count {n} exceeds static SK={SK}"
    kc_ = np.zeros((SK, DM), np.float16)
    kc_[:n] = k[b][sel]
    vc_ = np.zeros((SK, DM), np.float16)
    vc_[:n] = v[b][sel]
    mb = np.full(SK, MASK_BIAS, np.float32)
    mb[:n] = 0.0
    f16 = np.float16
    return {
        "qT": np.ascontiguousarray(q[b].T.astype(f16)),
        "kcT": np.ascontiguousarray(kc_.T),
        "vcT": np.ascontiguousarray(vc_.T),
        "wqT": np.ascontiguousarray(wq[hs * DEPTH:(hs + HPC) * DEPTH].T.astype(f16)),
        "wkT": np.ascontiguousarray(wk[hs * DEPTH:(hs + HPC) * DEPTH].T.astype(f16)),
        "wvT": np.ascontiguousarray(wv[hs * DEPTH:(hs + HPC) * DEPTH].T.astype(f16)),
        "fcT": np.ascontiguousarray(fc[:, hs * DEPTH:(hs + HPC) * DEPTH].T.astype(f16)),
        "maskb": np.ascontiguousarray(mb.reshape(KC, 128).T),
    }


def kernel(v, k, q, mask, wq, wk, wv, fc, _run_kwargs=None):
    v = np.asarray(v, np.float32)
    k = np.asarray(k, np.float32)
    q = np.asarray(q, np.float32)
    mask = np.asarray(mask)
    wq = np.asarray(wq, np.float32)
    wk = np.asarray(wk, np.float32)
    wv = np.asarray(wv, np.float32)
    fc = np.asarray(fc, np.float32)

    nc = _get_nc()
    in_maps = [_in_map_for_core(c, v, k, q, mask, wq, wk, wv, fc)
               for c in range(NCORES)]
    res = run_bass_kernel_spmd(nc, in_maps, core_ids=list(range(NCORES)),
                               **(_run_kwargs or {}))
    outs = [r["o"] for r in res.results]
    full = np.stack([outs[2 * b] + outs[2 * b + 1] for b in range(B)])
    if _run_kwargs:
        kernel.last_results = res
    return full


# revision 41
# speedup vs baseline: 1.0124x; 1.0124x over previous
"""Multi-head attention (B=4, S=2048, D=1024, H=16) on 8 trn2 NeuronCores.

Sharding: core c handles batch c//2 and heads (c%2)*8 .. (c%2)*8+8.
Each core computes its partial output through the fc projection; the host
sums the two per-batch partials.

Device dataflow (per core), everything fp16 inputs / fp32 accumulate:
  1. Project k, q into head-transposed layout  khT/qhT [c, token]
  2. Project v into  vhc [token, c]  with an appended ones column
  3. Per (head-pair, q-block): scores^T = kh^T q  [k, q] in PSUM (the two
     heads run concurrently in the PE array via row tiling), exp via ACT
     with a per-key bias (-50 for masked/padded keys, folding the
     key-padding mask), then P~^T + denominator via a [V | 1] matmul.
  4. Softmax denominators travel through DRAM, get reciprocated on a
     128-lane layout, broadcast back via partition-stride-0 DMA reads,
     and multiplied into P~^T -> ctxT.
  5. fc projection from ctxT -> partial output.

Keys are compacted on the host: masked keys (mask==1) are dropped and the
remainder zero-padded to SK=1152, cutting attention work ~44%.  The -50
exp-bias makes padded keys contribute exp(-50), which underflows to an
exact 0 in fp16.  All matmul operands are fp16 (host-cast; 10-bit
mantissa) with fp32 PSUM accumulation; softmax denominators and the
normalize stay fp32.  End-to-end scale-relative error ~6e-4.

Projection / fc work is interleaved into the attention loops as "filler"
so the PE array activity stays high (keeps the HAM clock gate at 2.4 GHz)
and no engine sits idle between phases.
"""

import numpy as np

import concourse.bass as bass
import concourse.tile as tile
from concourse import mybir
from concourse.bass_utils import run_bass_kernel_spmd

B, S, DM = 4, 2048, 1024
NH, DEPTH = 16, 64
NCORES = 8
HPC = 8                 # heads per core
C = HPC * DEPTH         # 512 output channels per core
SK = 1152               # compacted+padded key count
KC = SK // 128          # 9 key chunks
QW = 1024               # q-block width (bf16 moving limit)
NQW = S // QW           # 2
DC = DM // 128          # 8 contraction chunks
NPAIR = HPC // 2        # 4 head pairs (= c-tiles of 128)
SCALE = 1.0 / 8.0       # 1/sqrt(depth)
MASK_BIAS = -50.0

F32 = mybir.dt.float32
F32R = mybir.dt.float32r
BF16 = mybir.dt.bfloat16
FP16 = mybir.dt.float16
EXP = mybir.ActivationFunctionType.Exp


def _split_excess_waits(nc, cap_default=1, cap_evsem=2):
    """walrus in this env rejects >1 sync wait per instruction (2 for event
    semaphores); hoist excess waits onto preceding same-engine NoOps."""
    n_split = 0
    for f in nc.m.functions:
        for bb in f.blocks:
            insts = list(bb.instructions)
            out = []
            for inst in insts:
                si = inst.sync_info
                cap = cap_evsem if isinstance(inst, mybir.InstEventSemaphore) else cap_default
                if si is not None and si.on_wait and len(si.on_wait) > cap:
                    waits = list(si.on_wait)
                    extra, keep = waits[:-cap], waits[-cap:]
                    for i, w in enumerate(extra):
                        nop = mybir.InstNoOp(
                            name=f"{inst.name}_waitsplit_{i}",
                            sync_info=mybir.SyncInfo(on_wait=[w], on_update=[]),
                            bass_nofuse=True,
                            engine=inst.engine,
                        )
                        nc.register_instruction(nop, overwrite=True)
                        out.append(nop)
                    inst.sync_info = mybir.SyncInfo(on_wait=keep, on_update=list(si.on_update))
                    n_split += 1
                out.append(inst)
            if n_split:
                bb.instructions = out
    return n_split


def _emit(tc, t):
    nc = tc.nc
    from contextlib import ExitStack
    ctx = ExitStack()

    persist = ctx.enter_context(tc.tile_pool(name="persist", bufs=1))
    p_xrv = ctx.enter_context(tc.tile_pool(name="xrv", bufs=2))
    p_a = ctx.enter_context(tc.tile_pool(name="apool", bufs=8))
    p_dinvb = ctx.enter_context(tc.tile_pool(name="dinvb", bufs=2))
    p_small = ctx.enter_context(tc.tile_pool(name="small", bufs=2))
    p_fcr = ctx.enter_context(tc.tile_pool(name="fcr", bufs=8))
    p_out = ctx.enter_context(tc.tile_pool(name="outsb", bufs=2))
    p_s = ctx.enter_context(tc.tile_pool(name="pss", bufs=4, space="PSUM"))
    p_pv = ctx.enter_context(tc.tile_pool(name="pspv", bufs=3, space="PSUM"))
    p_pj = ctx.enter_context(tc.tile_pool(name="pspj", bufs=1, space="PSUM"))

    # persistent buffers
    wq_r = persist.tile([128, DC, C], FP16, tag="wq")
    wk_r = persist.tile([128, DC, C], FP16, tag="wk")
    wv_r = persist.tile([128, DC, C], FP16, tag="wv")
    xq_r = persist.tile([128, DC, S], FP16, tag="xq")
    xk_r = persist.tile([128, DC, SK], FP16, tag="xk")
    qhT = persist.tile([128, NPAIR, S], FP16, tag="qhT")
    khT = persist.tile([128, NPAIR, SK], FP16, tag="khT")
    vhc = persist.tile([128, KC, HPC, DEPTH + 1], FP16, tag="vhc")
    ctxT = persist.tile([128, NPAIR, S], FP16, tag="ctxT")
    maskb = persist.tile([128, KC], F32, tag="maskb")
    ones1 = persist.tile([128, 1], F32, tag="ones1")

    # internal DRAM for the denominator shuttle: row r=pair*32+hh*16+qq of 128
    d_dram = nc.dram_tensor("d_dram", (NPAIR * 32, 128), F32, kind="Internal").ap()
    dinv_dram = nc.dram_tensor("dinv_dram", (NPAIR * 32, 128), F32, kind="Internal").ap()
    dinv_flat = dinv_dram.rearrange("a b -> (a b)")
    # [pair, hh, qw, 1024] view for the per-(pair,qb) D-row writes
    d_view4 = d_dram.rearrange("(pr h q j) f -> pr h q (j f)", h=2, q=NQW, j=QW // 128)

    nc.sync.dma_start(maskb[:], t["maskb"])
    nc.vector.memset(ones1[:], 1.0)
    nc.vector.tensor_copy(
        vhc[:, :, :, DEPTH:DEPTH + 1],
        ones1[:].to_broadcast([128, KC, HPC, 1]),
    )

    # ---- weight + input loads (already fp16 from the host), spread
    # across four engines' DMA queues so the startup isn't issue-bound ----
    engs = [nc.sync, nc.scalar, nc.gpsimd]
    ei = 0

    def load(dst_ap, src_ap):
        nonlocal ei
        engs[ei % 3].dma_start(dst_ap, src_ap)
        ei += 1

    wk_v = t["wkT"].rearrange("(dc p) c -> p dc c", p=128)
    wq_v = t["wqT"].rearrange("(dc p) c -> p dc c", p=128)
    wv_v = t["wvT"].rearrange("(dc p) c -> p dc c", p=128)
    xk_v = t["kcT"].rearrange("(dc p) s -> p dc s", p=128)
    xq_v = t["qT"].rearrange("(dc p) s -> p dc s", p=128)
    # criticality order: wk + xk feed the first projection, then wq + xq,
    # wv last (v-projection runs after proj_qk(0))
    load(wk_r[:], wk_v[:])
    for dc in range(DC):
        load(xk_r[:, dc, :], xk_v[:, dc, :])
    load(wq_r[:], wq_v[:])
    for dc in range(DC):
        load(xq_r[:, dc, :], xq_v[:, dc, :])
    load(wv_r[:], wv_v[:])

    def proj_qk_tasks(pair):
        """Closures, each projecting one 512-token block of c-tile `pair`."""
        tasks = []
        for x_r, w_r, dst, slen in ((xk_r, wk_r, khT, SK), (xq_r, wq_r, qhT, S)):
            for tb0 in range(0, slen, 512):
                tlen = min(512, slen - tb0)

                def task(x_r=x_r, w_r=w_r, dst=dst, tb0=tb0, tlen=tlen, pair=pair):
                    ps = p_pj.tile([128, 512], F32, tag="pj",
                                   name=f"pj_{pair}_{dst.name}_{tb0}")
                    for dc in range(DC):
                        nc.tensor.matmul(ps[:, :tlen],
                                         w_r[:, dc, pair * 128:(pair + 1) * 128],
                                         x_r[:, dc, tb0:tb0 + tlen],
                                         start=(dc == 0), stop=(dc == DC - 1))
                    nc.vector.tensor_copy(dst[:, pair, tb0:tb0 + tlen], ps[:, :tlen])
                tasks.append(task)
        return tasks

    def proj_v():
        vview = t["vcT"].rearrange("(dc p) s -> p dc s", p=128)
        for kt in range(KC):
            xrv = p_xrv.tile([128, DC, 128], FP16, tag="xrv", name=f"xrv_{kt}")
            nc.sync.dma_start(xrv[:], vview[:, :, kt * 128:(kt + 1) * 128])
            ps = p_pj.tile([128, 512], F32, tag="pj", name=f"psv_{kt}")
            for dc in range(DC):
                nc.tensor.matmul(ps[:, :C], xrv[:, dc, :], wv_r[:, dc, :],
                                 start=(dc == 0), stop=(dc == DC - 1))
            nc.vector.tensor_copy(
                vhc[:, kt, :, 0:DEPTH],
                ps[:, :C].rearrange("p (h d) -> p h d", h=HPC),
            )

    def attention(pair, fillers, qw_order=None):
        """fillers: {qw: closures} consumed evenly across that qw's steps."""
        steps = 2 * KC
        for qw in (qw_order or range(NQW)):
            filler = fillers.get(qw, [])
            n_fill = len(filler)
            step = 0
            dst_stage = p_dinvb.tile([64, QW], F32, tag="dinvb",
                                     name=f"dstage_{pair}_{qw}")
            for sh in range(2):
                q0 = qw * QW + sh * 512
                pv = [p_pv.tile([DEPTH + 1, 512], F32, tag="pv",
                                name=f"pv_{pair}_{qw}_{sh}_{hh}") for hh in range(2)]
                a_prev = None
                for kc in range(KC):
                    if (n_fill and
                            step * n_fill // steps != (step + 1) * n_fill // steps):
                        filler[step * n_fill // steps]()
                    step += 1
                    cur = []
                    for hh in range(2):
                        lo = 64 * hh
                        ps_s = p_s.tile([128, 512], F32, tag="s",
                                        name=f"s_{pair}_{qw}_{sh}_{kc}_{hh}")
                        nc.tensor.matmul(ps_s[:], khT[lo:lo + 64, pair, kc * 128:(kc + 1) * 128],
                                         qhT[lo:lo + 64, pair, q0:q0 + 512],
                                         start=True, stop=True)
                        a_t = p_a.tile([128, 512], FP16, tag="A",
                                       name=f"A_{pair}_{qw}_{sh}_{kc}_{hh}")
                        nc.scalar.activation(a_t[:], ps_s[:], EXP,
                                             bias=maskb[:, kc:kc + 1], scale=SCALE)
                        cur.append(a_t)
                    if kc >= 1:
                        for hh in range(2):
                            nc.tensor.matmul(pv[hh][:], vhc[:, kc - 1, 2 * pair + hh, :],
                                             a_prev[hh][:], start=(kc == 1), stop=False)
                    a_prev = cur
                for hh in range(2):
                    nc.tensor.matmul(pv[hh][:], vhc[:, KC - 1, 2 * pair + hh, :],
                                     a_prev[hh][:], start=False, stop=True)
                for hh in range(2):
                    nc.vector.tensor_copy(dst_stage[32 * hh:32 * hh + 1, sh * 512:(sh + 1) * 512],
                                          pv[hh][DEPTH:DEPTH + 1, :])
                    nc.vector.tensor_copy(ctxT[64 * hh:64 * hh + 64, pair, q0:q0 + 512],
                                          pv[hh][0:DEPTH, :])
            for hh in range(2):
                nc.gpsimd.dma_start(d_view4[pair, hh:hh + 1, qw, :],
                                    dst_stage[32 * hh:32 * hh + 1, :])
            # denominator pipeline for this (pair, qw)
            d128 = p_small.tile([16, 128], F32, tag="d128", name=f"d128_{pair}_{qw}")
            for hh in range(2):
                r0 = pair * 32 + hh * 16 + qw * 8
                nc.sync.dma_start(d128[8 * hh:8 * hh + 8, :], d_dram[r0:r0 + 8, :])
            dinv = p_small.tile([16, 128], F32, tag="dinv", name=f"dinv_{pair}_{qw}")
            nc.vector.reciprocal(dinv[:], d128[:])
            for hh in range(2):
                r0 = pair * 32 + hh * 16 + qw * 8
                nc.sync.dma_start(dinv_dram[r0:r0 + 8, :], dinv[8 * hh:8 * hh + 8, :])
            db = p_dinvb.tile([128, QW], F32, tag="dinvb", name=f"db_{pair}_{qw}")
            for cc in range(2):
                off = pair * 4096 + cc * 2048 + qw * QW
                nc.gpsimd.dma_start(db[64 * cc:64 * cc + 64, :],
                                    dinv_flat[off:off + QW].partition_broadcast(64))
            for hh in range(2):
                sl = ctxT[64 * hh:64 * hh + 64, pair, qw * QW:(qw + 1) * QW]
                nc.vector.tensor_mul(sl, sl, db[64 * hh:64 * hh + 64, :])

    # ---- fc task construction (emission deferred) ----
    fc_view = t["fcT"].rearrange("(pr p) e -> p pr e", p=128)
    o_view = t["o"].rearrange("(tt p) e -> p tt e", p=128)
    fcrs = []
    for ec in range(2):
        for pair in range(NPAIR):
            fcr = p_fcr.tile([128, 512], FP16, tag="fcr", name=f"fcr_{ec}_{pair}")
            nc.sync.dma_start(fcr[:], fc_view[:, pair, ec * 512:(ec + 1) * 512])
            fcrs.append(fcr)

    def fc_task(tt, ec):
        def task():
            ps = p_pj.tile([128, 512], F32, tag="pj", name=f"fcps_{tt}_{ec}")
            for pair in range(NPAIR):
                nc.tensor.matmul(ps[:], ctxT[:, pair, tt * 128:(tt + 1) * 128],
                                 fcrs[ec * NPAIR + pair][:, :],
                                 start=(pair == 0), stop=(pair == NPAIR - 1))
            ob = p_out.tile([128, 512], F32, tag="outsb", name=f"ob_{tt}_{ec}")
            nc.vector.tensor_copy(ob[:], ps[:])
            nc.sync.dma_start(o_view[:, tt, ec * 512:(ec + 1) * 512], ob[:])
        return task

    # attention(3) runs qw=1 first; the fc groups that depend on qw=1
    # (tt 8..15) then fill its qw=0 half, and tt 0..7 run at the end.
    fc_fill = [fc_task(tt, ec) for tt in range(8, S // 128) for ec in range(2)]
    fc_tasks = [fc_task(tt, ec) for tt in range(8) for ec in range(2)]

    # ---- schedule: v-proj and pair-0 k/q proj up front, then per-pair
    # attention with the next pair's projections interleaved ----
    for task in proj_qk_tasks(0):
        task()
    proj_v()
    for pair in range(NPAIR):
        if pair + 1 < NPAIR:
            nxt = proj_qk_tasks(pair + 1)
            attention(pair, {0: nxt[:4], 1: nxt[4:]})
        else:
            attention(pair, {0: fc_fill}, qw_order=[1, 0])

    # ---- fc projection: 32 closures; first half fills attention(3) ----
    for task in fc_tasks:
        task()

    ctx.close()


_NC_CACHE = {}


def _get_nc():
    if "nc" in _NC_CACHE:
        return _NC_CACHE["nc"]
    nc = bass.Bass("TRN2", target_bir_lowering=False, debug=False)
    t = {
        "qT": nc.dram_tensor("qT", (DM, S), FP16, kind="ExternalInput").ap(),
        "kcT": nc.dram_tensor("kcT", (DM, SK), FP16, kind="ExternalInput").ap(),
        "vcT": nc.dram_tensor("vcT", (DM, SK), FP16, kind="ExternalInput").ap(),
        "wqT": nc.dram_tensor("wqT", (DM, C), FP16, kind="ExternalInput").ap(),
        "wkT": nc.dram_tensor("wkT", (DM, C), FP16, kind="ExternalInput").ap(),
        "wvT": nc.dram_tensor("wvT", (DM, C), FP16, kind="ExternalInput").ap(),
        "fcT": nc.dram_tensor("fcT", (C, DM), FP16, kind="ExternalInput").ap(),
        "maskb": nc.dram_tensor("maskb", (128, KC), F32, kind="ExternalInput").ap(),
        "o": nc.dram_tensor("o", (S, DM), F32, kind="ExternalOutput").ap(),
    }
    with tile.TileContext(nc) as tc:
        _emit(tc, t)
    _split_excess_waits(nc)
    _NC_CACHE["nc"] = nc
    return nc


def _in_map_for_core(core, v, k, q, mask, wq, wk, wv, fc):
    b = core // 2
    hs = (core % 2) * HPC
    sel = np.nonzero(mask[b] == 0)[0]
    n = len(sel)
    assert n <= SK, f"unmasked key count {n} exceeds static SK={SK}"
    kc_ = np.zeros((SK, DM), np.float16)
    kc_[:n] = k[b][sel]
    vc_ = np.zeros((SK, DM), np.float16)
    vc_[:n] = v[b][sel]
    mb = np.full(SK, MASK_BIAS, np.float32)
    mb[:n] = 0.0
    f16 = np.float16
    return {
        "qT": np.ascontiguousarray(q[b].T.astype(f16)),
        "kcT": np.ascontiguousarray(kc_.T),
        "vcT": np.ascontiguousarray(vc_.T),
        "wqT": np.ascontiguousarray(wq[hs * DEPTH:(hs + HPC) * DEPTH].T.astype(f16)),
        "wkT": np.ascontiguousarray(wk[hs * DEPTH:(hs + HPC) * DEPTH].T.astype(f16)),
        "wvT": np.ascontiguousarray(wv[hs * DEPTH:(hs + HPC) * DEPTH].T.astype(f16)),
        "fcT": np.ascontiguousarray(fc[:, hs * DEPTH:(hs + HPC) * DEPTH].T.astype(f16)),
        "maskb": np.ascontiguousarray(mb.reshape(KC, 128).T),
    }


def kernel(v, k, q, mask, wq, wk, wv, fc, _run_kwargs=None):
    v = np.asarray(v, np.float32)
    k = np.asarray(k, np.float32)
    q = np.asarray(q, np.float32)
    mask = np.asarray(mask)
    wq = np.asarray(wq, np.float32)
    wk = np.asarray(wk, np.float32)
    wv = np.asarray(wv, np.float32)
    fc = np.asarray(fc, np.float32)

    nc = _get_nc()
    in_maps = [_in_map_for_core(c, v, k, q, mask, wq, wk, wv, fc)
               for c in range(NCORES)]
    res = run_bass_kernel_spmd(nc, in_maps, core_ids=list(range(NCORES)),
                               **(_run_kwargs or {}))
    outs = [r["o"] for r in res.results]
    full = np.stack([outs[2 * b] + outs[2 * b + 1] for b in range(B)])
    if _run_kwargs:
        kernel.last_results = res
    return full


# revision 42
# speedup vs baseline: 1.0914x; 1.0780x over previous
"""Multi-head attention (B=4, S=2048, D=1024, H=16) on 8 trn2 NeuronCores.

Sharding: core c handles batch c//2 and heads (c%2)*8 .. (c%2)*8+8.
Each core computes its partial output through the fc projection; the host
sums the two per-batch partials.

Device dataflow (per core), everything fp16 inputs / fp32 accumulate:
  1. Project k, q into head-transposed layout  khT/qhT [c, token]
  2. Project v into  vhc [token, c]  with an appended ones column
  3. Per (head-pair, q-block): scores^T = kh^T q  [k, q] in PSUM (the two
     heads run concurrently in the PE array via row tiling), exp via ACT
     with a per-key bias (-50 for masked/padded keys, folding the
     key-padding mask), then P~^T + denominator via a [V | 1] matmul.
  4. Softmax denominators travel through DRAM, get reciprocated on a
     128-lane layout, broadcast back via partition-stride-0 DMA reads,
     and multiplied into P~^T -> ctxT.
  5. fc projection from ctxT -> partial output.

Keys are compacted on the host: masked keys (mask==1) are dropped and the
remainder zero-padded to SK=1152, cutting attention work ~44%.  The -50
exp-bias makes padded keys contribute exp(-50), which underflows to an
exact 0 in fp16.  All matmul operands are fp16 (host-cast; 10-bit
mantissa) with fp32 PSUM accumulation; softmax denominators and the
normalize stay fp32.  End-to-end scale-relative error ~6e-4.

Projection / fc work is interleaved into the attention loops as "filler"
so the PE array activity stays high (keeps the HAM clock gate at 2.4 GHz)
and no engine sits idle between phases.
"""

import numpy as np

import concourse.bass as bass
import concourse.tile as tile
from concourse import mybir
from concourse.bass_utils import run_bass_kernel_spmd

B, S, DM = 4, 2048, 1024
NH, DEPTH = 16, 64
NCORES = 8
HPC = 8                 # heads per core
C = HPC * DEPTH         # 512 output channels per core
SK = 1152               # compacted+padded key count
KC = SK // 128          # 9 key chunks
QW = 1024               # q-block width (bf16 moving limit)
NQW = S // QW           # 2
DC = DM // 128          # 8 contraction chunks
NPAIR = HPC // 2        # 4 head pairs (= c-tiles of 128)
SCALE = 1.0 / 8.0       # 1/sqrt(depth)
MASK_BIAS = -50.0

F32 = mybir.dt.float32
F32R = mybir.dt.float32r
BF16 = mybir.dt.bfloat16
FP16 = mybir.dt.float16
EXP = mybir.ActivationFunctionType.Exp


def _split_excess_waits(nc, cap_default=1, cap_evsem=2):
    """walrus in this env rejects >1 sync wait per instruction (2 for event
    semaphores); hoist excess waits onto preceding same-engine NoOps."""
    n_split = 0
    for f in nc.m.functions:
        for bb in f.blocks:
            insts = list(bb.instructions)
            out = []
            for inst in insts:
                si = inst.sync_info
                cap = cap_evsem if isinstance(inst, mybir.InstEventSemaphore) else cap_default
                if si is not None and si.on_wait and len(si.on_wait) > cap:
                    waits = list(si.on_wait)
                    extra, keep = waits[:-cap], waits[-cap:]
                    for i, w in enumerate(extra):
                        nop = mybir.InstNoOp(
                            name=f"{inst.name}_waitsplit_{i}",
                            sync_info=mybir.SyncInfo(on_wait=[w], on_update=[]),
                            bass_nofuse=True,
                            engine=inst.engine,
                        )
                        nc.register_instruction(nop, overwrite=True)
                        out.append(nop)
                    inst.sync_info = mybir.SyncInfo(on_wait=keep, on_update=list(si.on_update))
                    n_split += 1
                out.append(inst)
            if n_split:
                bb.instructions = out
    return n_split


def _emit(tc, t):
    nc = tc.nc
    from contextlib import ExitStack
    ctx = ExitStack()

    persist = ctx.enter_context(tc.tile_pool(name="persist", bufs=1))
    p_xrv = ctx.enter_context(tc.tile_pool(name="xrv", bufs=2))
    p_a = ctx.enter_context(tc.tile_pool(name="apool", bufs=8))
    p_dinvb = ctx.enter_context(tc.tile_pool(name="dinvb", bufs=2))
    p_small = ctx.enter_context(tc.tile_pool(name="small", bufs=2))
    p_fcr = ctx.enter_context(tc.tile_pool(name="fcr", bufs=8))
    p_out = ctx.enter_context(tc.tile_pool(name="outsb", bufs=4))
    p_s = ctx.enter_context(tc.tile_pool(name="pss", bufs=4, space="PSUM"))
    p_pv = ctx.enter_context(tc.tile_pool(name="pspv", bufs=3, space="PSUM"))
    p_pj = ctx.enter_context(tc.tile_pool(name="pspj", bufs=1, space="PSUM"))

    # persistent buffers
    wq_r = persist.tile([128, DC, C], FP16, tag="wq")
    wk_r = persist.tile([128, DC, C], FP16, tag="wk")
    wv_r = persist.tile([128, DC, C], FP16, tag="wv")
    xq_r = persist.tile([128, DC, S], FP16, tag="xq")
    xk_r = persist.tile([128, DC, SK], FP16, tag="xk")
    qhT = persist.tile([128, NPAIR, S], FP16, tag="qhT")
    khT = persist.tile([128, NPAIR, SK], FP16, tag="khT")
    vhc = persist.tile([128, KC, HPC, DEPTH + 1], FP16, tag="vhc")
    ctxT = persist.tile([128, NPAIR, S], FP16, tag="ctxT")
    maskb = persist.tile([128, KC], F32, tag="maskb")
    ones1 = persist.tile([128, 1], F32, tag="ones1")

    # internal DRAM for the denominator shuttle: row r=pair*32+hh*16+qq of 128
    d_dram = nc.dram_tensor("d_dram", (NPAIR * 32, 128), F32, kind="Internal").ap()
    dinv_dram = nc.dram_tensor("dinv_dram", (NPAIR * 32, 128), F32, kind="Internal").ap()
    dinv_flat = dinv_dram.rearrange("a b -> (a b)")
    # [pair, hh, qw, 1024] view for the per-(pair,qb) D-row writes
    d_view4 = d_dram.rearrange("(pr h q j) f -> pr h q (j f)", h=2, q=NQW, j=QW // 128)

    nc.sync.dma_start(maskb[:], t["maskb"])
    nc.vector.memset(ones1[:], 1.0)
    nc.vector.tensor_copy(
        vhc[:, :, :, DEPTH:DEPTH + 1],
        ones1[:].to_broadcast([128, KC, HPC, 1]),
    )

    # ---- weight + input loads (already fp16 from the host), spread
    # across four engines' DMA queues so the startup isn't issue-bound ----
    engs = [nc.sync, nc.scalar, nc.gpsimd]
    ei = 0

    def load(dst_ap, src_ap):
        nonlocal ei
        engs[ei % 3].dma_start(dst_ap, src_ap)
        ei += 1

    wk_v = t["wkT"].rearrange("(dc p) c -> p dc c", p=128)
    wq_v = t["wqT"].rearrange("(dc p) c -> p dc c", p=128)
    wv_v = t["wvT"].rearrange("(dc p) c -> p dc c", p=128)
    xk_v = t["kcT"].rearrange("(dc p) s -> p dc s", p=128)
    xq_v = t["qT"].rearrange("(dc p) s -> p dc s", p=128)
    # criticality order: wk + xk feed the first projection, then wq + xq,
    # wv last (v-projection runs after proj_qk(0))
    load(wk_r[:], wk_v[:])
    for dc in range(DC):
        load(xk_r[:, dc, :], xk_v[:, dc, :])
    load(wq_r[:], wq_v[:])
    for dc in range(DC):
        load(xq_r[:, dc, :], xq_v[:, dc, :])
    load(wv_r[:], wv_v[:])

    def proj_qk_tasks(pair):
        """Closures, each projecting one 512-token block of c-tile `pair`."""
        tasks = []
        for x_r, w_r, dst, slen in ((xk_r, wk_r, khT, SK), (xq_r, wq_r, qhT, S)):
            for tb0 in range(0, slen, 512):
                tlen = min(512, slen - tb0)

                def task(x_r=x_r, w_r=w_r, dst=dst, tb0=tb0, tlen=tlen, pair=pair):
                    ps = p_pj.tile([128, 512], F32, tag="pj",
                                   name=f"pj_{pair}_{dst.name}_{tb0}")
                    for dc in range(DC):
                        nc.tensor.matmul(ps[:, :tlen],
                                         w_r[:, dc, pair * 128:(pair + 1) * 128],
                                         x_r[:, dc, tb0:tb0 + tlen],
                                         start=(dc == 0), stop=(dc == DC - 1))
                    nc.vector.tensor_copy(dst[:, pair, tb0:tb0 + tlen], ps[:, :tlen])
                tasks.append(task)
        return tasks

    def proj_v():
        vview = t["vcT"].rearrange("(dc p) s -> p dc s", p=128)
        for kt in range(KC):
            xrv = p_xrv.tile([128, DC, 128], FP16, tag="xrv", name=f"xrv_{kt}")
            nc.sync.dma_start(xrv[:], vview[:, :, kt * 128:(kt + 1) * 128])
            ps = p_pj.tile([128, 512], F32, tag="pj", name=f"psv_{kt}")
            for dc in range(DC):
                nc.tensor.matmul(ps[:, :C], xrv[:, dc, :], wv_r[:, dc, :],
                                 start=(dc == 0), stop=(dc == DC - 1))
            nc.vector.tensor_copy(
                vhc[:, kt, :, 0:DEPTH],
                ps[:, :C].rearrange("p (h d) -> p h d", h=HPC),
            )

    def attention(pair, fillers, qw_order=None):
        """fillers: {qw: closures} consumed evenly across that qw's steps."""
        steps = 2 * KC
        for qw in (qw_order or range(NQW)):
            filler = fillers.get(qw, [])
            n_fill = len(filler)
            step = 0
            dst_stage = p_dinvb.tile([64, QW], F32, tag="dinvb",
                                     name=f"dstage_{pair}_{qw}")
            for sh in range(2):
                q0 = qw * QW + sh * 512
                pv = [p_pv.tile([DEPTH + 1, 512], F32, tag="pv",
                                name=f"pv_{pair}_{qw}_{sh}_{hh}") for hh in range(2)]
                a_prev = None
                for kc in range(KC):
                    if (n_fill and
                            step * n_fill // steps != (step + 1) * n_fill // steps):
                        filler[step * n_fill // steps]()
                    step += 1
                    cur = []
                    for hh in range(2):
                        lo = 64 * hh
                        ps_s = p_s.tile([128, 512], F32, tag="s",
                                        name=f"s_{pair}_{qw}_{sh}_{kc}_{hh}")
                        nc.tensor.matmul(ps_s[:], khT[lo:lo + 64, pair, kc * 128:(kc + 1) * 128],
                                         qhT[lo:lo + 64, pair, q0:q0 + 512],
                                         start=True, stop=True)
                        a_t = p_a.tile([128, 512], FP16, tag="A",
                                       name=f"A_{pair}_{qw}_{sh}_{kc}_{hh}")
                        nc.scalar.activation(a_t[:], ps_s[:], EXP,
                                             bias=maskb[:, kc:kc + 1], scale=SCALE)
                        cur.append(a_t)
                    if kc >= 1:
                        for hh in range(2):
                            nc.tensor.matmul(pv[hh][:], vhc[:, kc - 1, 2 * pair + hh, :],
                                             a_prev[hh][:], start=(kc == 1), stop=False)
                    a_prev = cur
                for hh in range(2):
                    nc.tensor.matmul(pv[hh][:], vhc[:, KC - 1, 2 * pair + hh, :],
                                     a_prev[hh][:], start=False, stop=True)
                for hh in range(2):
                    nc.vector.tensor_copy(dst_stage[32 * hh:32 * hh + 1, sh * 512:(sh + 1) * 512],
                                          pv[hh][DEPTH:DEPTH + 1, :])
                    nc.vector.tensor_copy(ctxT[64 * hh:64 * hh + 64, pair, q0:q0 + 512],
                                          pv[hh][0:DEPTH, :])
            for hh in range(2):
                nc.gpsimd.dma_start(d_view4[pair, hh:hh + 1, qw, :],
                                    dst_stage[32 * hh:32 * hh + 1, :])
            # denominator pipeline for this (pair, qw)
            d128 = p_small.tile([16, 128], F32, tag="d128", name=f"d128_{pair}_{qw}")
            for hh in range(2):
                r0 = pair * 32 + hh * 16 + qw * 8
                nc.sync.dma_start(d128[8 * hh:8 * hh + 8, :], d_dram[r0:r0 + 8, :])
            dinv = p_small.tile([16, 128], F32, tag="dinv", name=f"dinv_{pair}_{qw}")
            nc.vector.reciprocal(dinv[:], d128[:])
            for hh in range(2):
                r0 = pair * 32 + hh * 16 + qw * 8
                nc.sync.dma_start(dinv_dram[r0:r0 + 8, :], dinv[8 * hh:8 * hh + 8, :])
            db = p_dinvb.tile([128, QW], F32, tag="dinvb", name=f"db_{pair}_{qw}")
            for cc in range(2):
                off = pair * 4096 + cc * 2048 + qw * QW
                nc.gpsimd.dma_start(db[64 * cc:64 * cc + 64, :],
                                    dinv_flat[off:off + QW].partition_broadcast(64))
            for hh in range(2):
                sl = ctxT[64 * hh:64 * hh + 64, pair, qw * QW:(qw + 1) * QW]
                nc.vector.tensor_mul(sl, sl, db[64 * hh:64 * hh + 64, :])

    # ---- fc task construction (emission deferred) ----
    fc_view = t["fcT"].rearrange("(pr p) e -> p pr e", p=128)
    o_view = t["o"].rearrange("(tt p) e -> p tt e", p=128)
    fcrs = []
    for ec in range(2):
        for pair in range(NPAIR):
            fcr = p_fcr.tile([128, 512], FP16, tag="fcr", name=f"fcr_{ec}_{pair}")
            nc.sync.dma_start(fcr[:], fc_view[:, pair, ec * 512:(ec + 1) * 512])
            fcrs.append(fcr)

    def fc_task(tt, ec, tail=False):
        def task():
            # tail groups run after attention: rotate through the then-idle
            # 4-slot score pool and copy via the idle ACT engine, so the
            # matmul groups stream instead of serializing on one bank
            pool, tag = (p_s, "s") if tail else (p_pj, "pj")
            ps = pool.tile([128, 512], F32, tag=tag, name=f"fcps_{tt}_{ec}")
            for pair in range(NPAIR):
                nc.tensor.matmul(ps[:], ctxT[:, pair, tt * 128:(tt + 1) * 128],
                                 fcrs[ec * NPAIR + pair][:, :],
                                 start=(pair == 0), stop=(pair == NPAIR - 1))
            ob = p_out.tile([128, 512], F32, tag="outsb", name=f"ob_{tt}_{ec}")
            if tail:
                nc.scalar.copy(ob[:], ps[:])
            else:
                nc.vector.tensor_copy(ob[:], ps[:])
            nc.sync.dma_start(o_view[:, tt, ec * 512:(ec + 1) * 512], ob[:])
        return task

    # attention(3) runs qw=1 first; the fc groups that depend on qw=1
    # (tt 8..15) then fill its qw=0 half, and tt 0..7 run at the end.
    fc_fill = [fc_task(tt, ec) for tt in range(8, S // 128) for ec in range(2)]
    fc_tasks = [fc_task(tt, ec, tail=True) for tt in range(8) for ec in range(2)]

    # ---- schedule: v-proj and pair-0 k/q proj up front, then per-pair
    # attention with the next pair's projections interleaved ----
    for task in proj_qk_tasks(0):
        task()
    proj_v()
    for pair in range(NPAIR):
        if pair + 1 < NPAIR:
            nxt = proj_qk_tasks(pair + 1)
            attention(pair, {0: nxt[:4], 1: nxt[4:]})
        else:
            attention(pair, {0: fc_fill}, qw_order=[1, 0])

    # ---- fc projection: 32 closures; first half fills attention(3) ----
    for task in fc_tasks:
        task()

    ctx.close()


_NC_CACHE = {}


def _get_nc():
    if "nc" in _NC_CACHE:
        return _NC_CACHE["nc"]
    nc = bass.Bass("TRN2", target_bir_lowering=False, debug=False)
    t = {
        "qT": nc.dram_tensor("qT", (DM, S), FP16, kind="ExternalInput").ap(),
        "kcT": nc.dram_tensor("kcT", (DM, SK), FP16, kind="ExternalInput").ap(),
        "vcT": nc.dram_tensor("vcT", (DM, SK), FP16, kind="ExternalInput").ap(),
        "wqT": nc.dram_tensor("wqT", (DM, C), FP16, kind="ExternalInput").ap(),
        "wkT": nc.dram_tensor("wkT", (DM, C), FP16, kind="ExternalInput").ap(),
        "wvT": nc.dram_tensor("wvT", (DM, C), FP16, kind="ExternalInput").ap(),
        "fcT": nc.dram_tensor("fcT", (C, DM), FP16, kind="ExternalInput").ap(),
        "maskb": nc.dram_tensor("maskb", (128, KC), F32, kind="ExternalInput").ap(),
        "o": nc.dram_tensor("o", (S, DM), F32, kind="ExternalOutput").ap(),
    }
    with tile.TileContext(nc) as tc:
        _emit(tc, t)
    _split_excess_waits(nc)
    _NC_CACHE["nc"] = nc
    return nc


def _in_map_for_core(core, v, k, q, mask, wq, wk, wv, fc):
    b = core // 2
    hs = (core % 2) * HPC
    sel = np.nonzero(mask[b] == 0)[0]
    n = len(sel)
    assert n <= SK, f"unmasked key count {n} exceeds static SK={SK}"
    kc_ = np.zeros((SK, DM), np.float16)
    kc_[:n] = k[b][sel]
    vc_ = np.zeros((SK, DM), np.float16)
    vc_[:n] = v[b][sel]
    mb = np.full(SK, MASK_BIAS, np.float32)
    mb[:n] = 0.0
    f16 = np.float16
    return {
        "qT": np.ascontiguousarray(q[b].T.astype(f16)),
        "kcT": np.ascontiguousarray(kc_.T),
        "vcT": np.ascontiguousarray(vc_.T),
        "wqT": np.ascontiguousarray(wq[hs * DEPTH:(hs + HPC) * DEPTH].T.astype(f16)),
        "wkT": np.ascontiguousarray(wk[hs * DEPTH:(hs + HPC) * DEPTH].T.astype(f16)),
        "wvT": np.ascontiguousarray(wv[hs * DEPTH:(hs + HPC) * DEPTH].T.astype(f16)),
        "fcT": np.ascontiguousarray(fc[:, hs * DEPTH:(hs + HPC) * DEPTH].T.astype(f16)),
        "maskb": np.ascontiguousarray(mb.reshape(KC, 128).T),
    }


def kernel(v, k, q, mask, wq, wk, wv, fc, _run_kwargs=None):
    v = np.asarray(v, np.float32)
    k = np.asarray(k, np.float32)
    q = np.asarray(q, np.float32)
    mask = np.asarray(mask)
    wq = np.asarray(wq, np.float32)
    wk = np.asarray(wk, np.float32)
    wv = np.asarray(wv, np.float32)
    fc = np.asarray(fc, np.float32)

    nc = _get_nc()
    in_maps = [_in_map_for_core(c, v, k, q, mask, wq, wk, wv, fc)
               for c in range(NCORES)]
    res = run_bass_kernel_spmd(nc, in_maps, core_ids=list(range(NCORES)),
                               **(_run_kwargs or {}))
    outs = [r["o"] for r in res.results]
    full = np.stack([outs[2 * b] + outs[2 * b + 1] for b in range(B)])
    if _run_kwargs:
        kernel.last_results = res
    return full


# revision 43
# speedup vs baseline: 1.1069x; 1.0142x over previous
"""Multi-head attention (B=4, S=2048, D=1024, H=16) on 8 trn2 NeuronCores.

Sharding: core c handles batch c//2 and heads (c%2)*8 .. (c%2)*8+8.
Each core computes its partial output through the fc projection; the host
sums the two per-batch partials.

Device dataflow (per core), everything fp16 inputs / fp32 accumulate:
  1. Project k, q into head-transposed layout  khT/qhT [c, token]
  2. Project v into  vhc [token, c]  with an appended ones column
  3. Per (head-pair, q-block): scores^T = kh^T q  [k, q] in PSUM (the two
     heads run concurrently in the PE array via row tiling), exp via ACT
     with a per-key bias (-50 for masked/padded keys, folding the
     key-padding mask), then P~^T + denominator via a [V | 1] matmul.
  4. Softmax denominators travel through DRAM, get reciprocated on a
     128-lane layout, broadcast back via partition-stride-0 DMA reads,
     and multiplied into P~^T -> ctxT.
  5. fc projection from ctxT -> partial output.

Keys are compacted on the host: masked keys (mask==1) are dropped and the
remainder zero-padded to SK=1152, cutting attention work ~44%.  The -50
exp-bias makes padded keys contribute exp(-50), which underflows to an
exact 0 in fp16.  All matmul operands are fp16 (host-cast; 10-bit
mantissa) with fp32 PSUM accumulation; softmax denominators and the
normalize stay fp32.  End-to-end scale-relative error ~6e-4.

Projection / fc work is interleaved into the attention loops as "filler"
so the PE array activity stays high (keeps the HAM clock gate at 2.4 GHz)
and no engine sits idle between phases.
"""

import numpy as np

import concourse.bass as bass
import concourse.tile as tile
from concourse import mybir
from concourse.bass_utils import run_bass_kernel_spmd

B, S, DM = 4, 2048, 1024
NH, DEPTH = 16, 64
NCORES = 8
HPC = 8                 # heads per core
C = HPC * DEPTH         # 512 output channels per core
SK = 1152               # compacted+padded key count
KC = SK // 128          # 9 key chunks
QW = 1024               # q-block width (bf16 moving limit)
NQW = S // QW           # 2
DC = DM // 128          # 8 contraction chunks
NPAIR = HPC // 2        # 4 head pairs (= c-tiles of 128)
SCALE = 1.0 / 8.0       # 1/sqrt(depth)
MASK_BIAS = -50.0

F32 = mybir.dt.float32
F32R = mybir.dt.float32r
BF16 = mybir.dt.bfloat16
FP16 = mybir.dt.float16
EXP = mybir.ActivationFunctionType.Exp


def _split_excess_waits(nc, cap_default=1, cap_evsem=2):
    """walrus in this env rejects >1 sync wait per instruction (2 for event
    semaphores); hoist excess waits onto preceding same-engine NoOps."""
    n_split = 0
    for f in nc.m.functions:
        for bb in f.blocks:
            insts = list(bb.instructions)
            out = []
            for inst in insts:
                si = inst.sync_info
                cap = cap_evsem if isinstance(inst, mybir.InstEventSemaphore) else cap_default
                if si is not None and si.on_wait and len(si.on_wait) > cap:
                    waits = list(si.on_wait)
                    extra, keep = waits[:-cap], waits[-cap:]
                    for i, w in enumerate(extra):
                        nop = mybir.InstNoOp(
                            name=f"{inst.name}_waitsplit_{i}",
                            sync_info=mybir.SyncInfo(on_wait=[w], on_update=[]),
                            bass_nofuse=True,
                            engine=inst.engine,
                        )
                        nc.register_instruction(nop, overwrite=True)
                        out.append(nop)
                    inst.sync_info = mybir.SyncInfo(on_wait=keep, on_update=list(si.on_update))
                    n_split += 1
                out.append(inst)
            if n_split:
                bb.instructions = out
    return n_split


def _emit(tc, t):
    nc = tc.nc
    from contextlib import ExitStack
    ctx = ExitStack()

    persist = ctx.enter_context(tc.tile_pool(name="persist", bufs=1))
    p_xrv = ctx.enter_context(tc.tile_pool(name="xrv", bufs=2))
    p_a = ctx.enter_context(tc.tile_pool(name="apool", bufs=8))
    p_dinvb = ctx.enter_context(tc.tile_pool(name="dinvb", bufs=2))
    p_small = ctx.enter_context(tc.tile_pool(name="small", bufs=2))
    p_fcr = ctx.enter_context(tc.tile_pool(name="fcr", bufs=8))
    p_out = ctx.enter_context(tc.tile_pool(name="outsb", bufs=4))
    p_s = ctx.enter_context(tc.tile_pool(name="pss", bufs=4, space="PSUM"))
    p_pv = ctx.enter_context(tc.tile_pool(name="pspv", bufs=3, space="PSUM"))
    p_pj = ctx.enter_context(tc.tile_pool(name="pspj", bufs=1, space="PSUM"))

    # persistent buffers
    wq_r = persist.tile([128, DC, C], FP16, tag="wq")
    wk_r = persist.tile([128, DC, C], FP16, tag="wk")
    wv_r = persist.tile([128, DC, C], FP16, tag="wv")
    xq_r = persist.tile([128, DC, S], FP16, tag="xq")
    xk_r = persist.tile([128, DC, SK], FP16, tag="xk")
    qhT = persist.tile([128, NPAIR, S], FP16, tag="qhT")
    khT = persist.tile([128, NPAIR, SK], FP16, tag="khT")
    vhc = persist.tile([128, KC, HPC, DEPTH + 1], FP16, tag="vhc")
    ctxT = persist.tile([128, NPAIR, S], FP16, tag="ctxT")
    maskb = persist.tile([128, KC], F32, tag="maskb")
    ones1 = persist.tile([128, 1], F32, tag="ones1")

    # internal DRAM for the denominator shuttle: row r=pair*32+hh*16+qq of 128
    d_dram = nc.dram_tensor("d_dram", (NPAIR * 32, 128), F32, kind="Internal").ap()
    dinv_dram = nc.dram_tensor("dinv_dram", (NPAIR * 32, 128), F32, kind="Internal").ap()
    dinv_flat = dinv_dram.rearrange("a b -> (a b)")
    # [pair, hh, qw, 1024] view for the per-(pair,qb) D-row writes
    d_view4 = d_dram.rearrange("(pr h q j) f -> pr h q (j f)", h=2, q=NQW, j=QW // 128)

    nc.sync.dma_start(maskb[:], t["maskb"])
    nc.vector.memset(ones1[:], 1.0)
    nc.vector.tensor_copy(
        vhc[:, :, :, DEPTH:DEPTH + 1],
        ones1[:].to_broadcast([128, KC, HPC, 1]),
    )

    # ---- weight + input loads (already fp16 from the host), spread
    # across four engines' DMA queues so the startup isn't issue-bound ----
    engs = [nc.sync, nc.scalar, nc.gpsimd]
    ei = 0

    def load(dst_ap, src_ap):
        nonlocal ei
        engs[ei % 3].dma_start(dst_ap, src_ap)
        ei += 1

    wk_v = t["wkT"].rearrange("(dc p) c -> p dc c", p=128)
    wq_v = t["wqT"].rearrange("(dc p) c -> p dc c", p=128)
    wv_v = t["wvT"].rearrange("(dc p) c -> p dc c", p=128)
    xk_v = t["kcT"].rearrange("(dc p) s -> p dc s", p=128)
    xq_v = t["qT"].rearrange("(dc p) s -> p dc s", p=128)
    # criticality order: wk + xk feed the first projection, then wq + xq,
    # wv last (v-projection runs after proj_qk(0))
    load(wk_r[:], wk_v[:])
    for dc in range(DC):
        load(xk_r[:, dc, :], xk_v[:, dc, :])
    load(wq_r[:], wq_v[:])
    for dc in range(DC):
        load(xq_r[:, dc, :], xq_v[:, dc, :])
    load(wv_r[:], wv_v[:])

    def proj_qk_tasks(pair):
        """Closures, each projecting one 512-token block of c-tile `pair`."""
        tasks = []
        for x_r, w_r, dst, slen in ((xk_r, wk_r, khT, SK), (xq_r, wq_r, qhT, S)):
            for tb0 in range(0, slen, 512):
                tlen = min(512, slen - tb0)

                def task(x_r=x_r, w_r=w_r, dst=dst, tb0=tb0, tlen=tlen, pair=pair):
                    # pair 0 projects before attention starts: the score pool
                    # is idle then, so rotate through its 4 banks instead of
                    # serializing on the single filler bank
                    pool, tag = (p_s, "s") if pair == 0 else (p_pj, "pj")
                    ps = pool.tile([128, 512], F32, tag=tag,
                                   name=f"pj_{pair}_{dst.name}_{tb0}")
                    for dc in range(DC):
                        nc.tensor.matmul(ps[:, :tlen],
                                         w_r[:, dc, pair * 128:(pair + 1) * 128],
                                         x_r[:, dc, tb0:tb0 + tlen],
                                         start=(dc == 0), stop=(dc == DC - 1))
                    nc.vector.tensor_copy(dst[:, pair, tb0:tb0 + tlen], ps[:, :tlen])
                tasks.append(task)
        return tasks

    def proj_v():
        vview = t["vcT"].rearrange("(dc p) s -> p dc s", p=128)
        for kt in range(KC):
            xrv = p_xrv.tile([128, DC, 128], FP16, tag="xrv", name=f"xrv_{kt}")
            nc.sync.dma_start(xrv[:], vview[:, :, kt * 128:(kt + 1) * 128])
            ps = p_s.tile([128, 512], F32, tag="s", name=f"psv_{kt}")
            for dc in range(DC):
                nc.tensor.matmul(ps[:, :C], xrv[:, dc, :], wv_r[:, dc, :],
                                 start=(dc == 0), stop=(dc == DC - 1))
            nc.vector.tensor_copy(
                vhc[:, kt, :, 0:DEPTH],
                ps[:, :C].rearrange("p (h d) -> p h d", h=HPC),
            )

    def attention(pair, fillers, qw_order=None):
        """fillers: {qw: closures} consumed evenly across that qw's steps."""
        steps = 2 * KC
        for qw in (qw_order or range(NQW)):
            filler = fillers.get(qw, [])
            n_fill = len(filler)
            step = 0
            dst_stage = p_dinvb.tile([64, QW], F32, tag="dinvb",
                                     name=f"dstage_{pair}_{qw}")
            for sh in range(2):
                q0 = qw * QW + sh * 512
                pv = [p_pv.tile([DEPTH + 1, 512], F32, tag="pv",
                                name=f"pv_{pair}_{qw}_{sh}_{hh}") for hh in range(2)]
                a_prev = None
                for kc in range(KC):
                    if (n_fill and
                            step * n_fill // steps != (step + 1) * n_fill // steps):
                        filler[step * n_fill // steps]()
                    step += 1
                    cur = []
                    for hh in range(2):
                        lo = 64 * hh
                        ps_s = p_s.tile([128, 512], F32, tag="s",
                                        name=f"s_{pair}_{qw}_{sh}_{kc}_{hh}")
                        nc.tensor.matmul(ps_s[:], khT[lo:lo + 64, pair, kc * 128:(kc + 1) * 128],
                                         qhT[lo:lo + 64, pair, q0:q0 + 512],
                                         start=True, stop=True)
                        a_t = p_a.tile([128, 512], FP16, tag="A",
                                       name=f"A_{pair}_{qw}_{sh}_{kc}_{hh}")
                        nc.scalar.activation(a_t[:], ps_s[:], EXP,
                                             bias=maskb[:, kc:kc + 1], scale=SCALE)
                        cur.append(a_t)
                    if kc >= 1:
                        for hh in range(2):
                            nc.tensor.matmul(pv[hh][:], vhc[:, kc - 1, 2 * pair + hh, :],
                                             a_prev[hh][:], start=(kc == 1), stop=False)
                    a_prev = cur
                for hh in range(2):
                    nc.tensor.matmul(pv[hh][:], vhc[:, KC - 1, 2 * pair + hh, :],
                                     a_prev[hh][:], start=False, stop=True)
                for hh in range(2):
                    nc.vector.tensor_copy(dst_stage[32 * hh:32 * hh + 1, sh * 512:(sh + 1) * 512],
                                          pv[hh][DEPTH:DEPTH + 1, :])
                    nc.vector.tensor_copy(ctxT[64 * hh:64 * hh + 64, pair, q0:q0 + 512],
                                          pv[hh][0:DEPTH, :])
            for hh in range(2):
                nc.gpsimd.dma_start(d_view4[pair, hh:hh + 1, qw, :],
                                    dst_stage[32 * hh:32 * hh + 1, :])
            # denominator pipeline for this (pair, qw)
            d128 = p_small.tile([16, 128], F32, tag="d128", name=f"d128_{pair}_{qw}")
            for hh in range(2):
                r0 = pair * 32 + hh * 16 + qw * 8
                nc.sync.dma_start(d128[8 * hh:8 * hh + 8, :], d_dram[r0:r0 + 8, :])
            dinv = p_small.tile([16, 128], F32, tag="dinv", name=f"dinv_{pair}_{qw}")
            nc.vector.reciprocal(dinv[:], d128[:])
            for hh in range(2):
                r0 = pair * 32 + hh * 16 + qw * 8
                nc.sync.dma_start(dinv_dram[r0:r0 + 8, :], dinv[8 * hh:8 * hh + 8, :])
            db = p_dinvb.tile([128, QW], F32, tag="dinvb", name=f"db_{pair}_{qw}")
            for cc in range(2):
                off = pair * 4096 + cc * 2048 + qw * QW
                nc.gpsimd.dma_start(db[64 * cc:64 * cc + 64, :],
                                    dinv_flat[off:off + QW].partition_broadcast(64))
            for hh in range(2):
                sl = ctxT[64 * hh:64 * hh + 64, pair, qw * QW:(qw + 1) * QW]
                nc.vector.tensor_mul(sl, sl, db[64 * hh:64 * hh + 64, :])

    # ---- fc task construction (emission deferred) ----
    fc_view = t["fcT"].rearrange("(pr p) e -> p pr e", p=128)
    o_view = t["o"].rearrange("(tt p) e -> p tt e", p=128)
    fcrs = []
    for ec in range(2):
        for pair in range(NPAIR):
            fcr = p_fcr.tile([128, 512], FP16, tag="fcr", name=f"fcr_{ec}_{pair}")
            nc.sync.dma_start(fcr[:], fc_view[:, pair, ec * 512:(ec + 1) * 512])
            fcrs.append(fcr)

    def fc_task(tt, ec, tail=False):
        def task():
            # tail groups run after attention: rotate through the then-idle
            # 4-slot score pool and copy via the idle ACT engine, so the
            # matmul groups stream instead of serializing on one bank
            pool, tag = (p_s, "s") if tail else (p_pj, "pj")
            ps = pool.tile([128, 512], F32, tag=tag, name=f"fcps_{tt}_{ec}")
            for pair in range(NPAIR):
                nc.tensor.matmul(ps[:], ctxT[:, pair, tt * 128:(tt + 1) * 128],
                                 fcrs[ec * NPAIR + pair][:, :],
                                 start=(pair == 0), stop=(pair == NPAIR - 1))
            ob = p_out.tile([128, 512], F32, tag="outsb", name=f"ob_{tt}_{ec}")
            if tail:
                nc.scalar.copy(ob[:], ps[:])
            else:
                nc.vector.tensor_copy(ob[:], ps[:])
            nc.sync.dma_start(o_view[:, tt, ec * 512:(ec + 1) * 512], ob[:])
        return task

    # attention(3) runs qw=1 first; the fc groups that depend on qw=1
    # (tt 8..15) then fill its qw=0 half, and tt 0..7 run at the end.
    fc_fill = [fc_task(tt, ec) for tt in range(8, S // 128) for ec in range(2)]
    fc_tasks = [fc_task(tt, ec, tail=True) for tt in range(8) for ec in range(2)]

    # ---- schedule: v-proj and pair-0 k/q proj up front, then per-pair
    # attention with the next pair's projections interleaved ----
    for task in proj_qk_tasks(0):
        task()
    proj_v()
    for pair in range(NPAIR):
        if pair + 1 < NPAIR:
            nxt = proj_qk_tasks(pair + 1)
            attention(pair, {0: nxt[:4], 1: nxt[4:]})
        else:
            attention(pair, {0: fc_fill}, qw_order=[1, 0])

    # ---- fc projection: 32 closures; first half fills attention(3) ----
    for task in fc_tasks:
        task()

    ctx.close()


_NC_CACHE = {}


def _get_nc():
    if "nc" in _NC_CACHE:
        return _NC_CACHE["nc"]
    nc = bass.Bass("TRN2", target_bir_lowering=False, debug=False)
    t = {
        "qT": nc.dram_tensor("qT", (DM, S), FP16, kind="ExternalInput").ap(),
        "kcT": nc.dram_tensor("kcT", (DM, SK), FP16, kind="ExternalInput").ap(),
        "vcT": nc.dram_tensor("vcT", (DM, SK), FP16, kind="ExternalInput").ap(),
        "wqT": nc.dram_tensor("wqT", (DM, C), FP16, kind="ExternalInput").ap(),
        "wkT": nc.dram_tensor("wkT", (DM, C), FP16, kind="ExternalInput").ap(),
        "wvT": nc.dram_tensor("wvT", (DM, C), FP16, kind="ExternalInput").ap(),
        "fcT": nc.dram_tensor("fcT", (C, DM), FP16, kind="ExternalInput").ap(),
        "maskb": nc.dram_tensor("maskb", (128, KC), F32, kind="ExternalInput").ap(),
        "o": nc.dram_tensor("o", (S, DM), F32, kind="ExternalOutput").ap(),
    }
    with tile.TileContext(nc) as tc:
        _emit(tc, t)
    _split_excess_waits(nc)
    _NC_CACHE["nc"] = nc
    return nc


def _in_map_for_core(core, v, k, q, mask, wq, wk, wv, fc):
    b = core // 2
    hs = (core % 2) * HPC
    sel = np.nonzero(mask[b] == 0)[0]
    n = len(sel)
    assert n <= SK, f"unmasked key count {n} exceeds static SK={SK}"
    kc_ = np.zeros((SK, DM), np.float16)
    kc_[:n] = k[b][sel]
    vc_ = np.zeros((SK, DM), np.float16)
    vc_[:n] = v[b][sel]
    mb = np.full(SK, MASK_BIAS, np.float32)
    mb[:n] = 0.0
    f16 = np.float16
    return {
        "qT": np.ascontiguousarray(q[b].T.astype(f16)),
        "kcT": np.ascontiguousarray(kc_.T),
        "vcT": np.ascontiguousarray(vc_.T),
        "wqT": np.ascontiguousarray(wq[hs * DEPTH:(hs + HPC) * DEPTH].T.astype(f16)),
        "wkT": np.ascontiguousarray(wk[hs * DEPTH:(hs + HPC) * DEPTH].T.astype(f16)),
        "wvT": np.ascontiguousarray(wv[hs * DEPTH:(hs + HPC) * DEPTH].T.astype(f16)),
        "fcT": np.ascontiguousarray(fc[:, hs * DEPTH:(hs + HPC) * DEPTH].T.astype(f16)),
        "maskb": np.ascontiguousarray(mb.reshape(KC, 128).T),
    }


def kernel(v, k, q, mask, wq, wk, wv, fc, _run_kwargs=None):
    v = np.asarray(v, np.float32)
    k = np.asarray(k, np.float32)
    q = np.asarray(q, np.float32)
    mask = np.asarray(mask)
    wq = np.asarray(wq, np.float32)
    wk = np.asarray(wk, np.float32)
    wv = np.asarray(wv, np.float32)
    fc = np.asarray(fc, np.float32)

    nc = _get_nc()
    in_maps = [_in_map_for_core(c, v, k, q, mask, wq, wk, wv, fc)
               for c in range(NCORES)]
    res = run_bass_kernel_spmd(nc, in_maps, core_ids=list(range(NCORES)),
                               **(_run_kwargs or {}))
    outs = [r["o"] for r in res.results]
    full = np.stack([outs[2 * b] + outs[2 * b + 1] for b in range(B)])
    if _run_kwargs:
        kernel.last_results = res
    return full


# revision 44
# speedup vs baseline: 1.1118x; 1.0044x over previous
"""Multi-head attention (B=4, S=2048, D=1024, H=16) on 8 trn2 NeuronCores.

Sharding: core c handles batch c//2 and heads (c%2)*8 .. (c%2)*8+8.
Each core computes its partial output through the fc projection; the host
sums the two per-batch partials.

Device dataflow (per core), everything fp16 inputs / fp32 accumulate:
  1. Project k, q into head-transposed layout  khT/qhT [c, token]
  2. Project v into  vhc [token, c]  with an appended ones column
  3. Per (head-pair, q-block): scores^T = kh^T q  [k, q] in PSUM (the two
     heads run concurrently in the PE array via row tiling), exp via ACT
     with a per-key bias (-50 for masked/padded keys, folding the
     key-padding mask), then P~^T + denominator via a [V | 1] matmul.
  4. Softmax denominators travel through DRAM, get reciprocated on a
     128-lane layout, broadcast back via partition-stride-0 DMA reads,
     and multiplied into P~^T -> ctxT.
  5. fc projection from ctxT -> partial output.

Keys are compacted on the host: masked keys (mask==1) are dropped and the
remainder zero-padded to SK=1152, cutting attention work ~44%.  The -50
exp-bias makes padded keys contribute exp(-50), which underflows to an
exact 0 in fp16.  All matmul operands are fp16 (host-cast; 10-bit
mantissa) with fp32 PSUM accumulation; softmax denominators and the
normalize stay fp32.  End-to-end scale-relative error ~6e-4.

Projection / fc work is interleaved into the attention loops as "filler"
so the PE array activity stays high (keeps the HAM clock gate at 2.4 GHz)
and no engine sits idle between phases.
"""

import numpy as np

import concourse.bass as bass
import concourse.tile as tile
from concourse import mybir
from concourse.bass_utils import run_bass_kernel_spmd

B, S, DM = 4, 2048, 1024
NH, DEPTH = 16, 64
NCORES = 8
HPC = 8                 # heads per core
C = HPC * DEPTH         # 512 output channels per core
SK = 1152               # compacted+padded key count
KC = SK // 128          # 9 key chunks
QW = 1024               # q-block width (bf16 moving limit)
NQW = S // QW           # 2
DC = DM // 128          # 8 contraction chunks
NPAIR = HPC // 2        # 4 head pairs (= c-tiles of 128)
SCALE = 1.0 / 8.0       # 1/sqrt(depth)
MASK_BIAS = -50.0

F32 = mybir.dt.float32
F32R = mybir.dt.float32r
BF16 = mybir.dt.bfloat16
FP16 = mybir.dt.float16
EXP = mybir.ActivationFunctionType.Exp


def _split_excess_waits(nc, cap_default=1, cap_evsem=2):
    """walrus in this env rejects >1 sync wait per instruction (2 for event
    semaphores); hoist excess waits onto preceding same-engine NoOps."""
    n_split = 0
    for f in nc.m.functions:
        for bb in f.blocks:
            insts = list(bb.instructions)
            out = []
            for inst in insts:
                si = inst.sync_info
                cap = cap_evsem if isinstance(inst, mybir.InstEventSemaphore) else cap_default
                if si is not None and si.on_wait and len(si.on_wait) > cap:
                    waits = list(si.on_wait)
                    extra, keep = waits[:-cap], waits[-cap:]
                    for i, w in enumerate(extra):
                        nop = mybir.InstNoOp(
                            name=f"{inst.name}_waitsplit_{i}",
                            sync_info=mybir.SyncInfo(on_wait=[w], on_update=[]),
                            bass_nofuse=True,
                            engine=inst.engine,
                        )
                        nc.register_instruction(nop, overwrite=True)
                        out.append(nop)
                    inst.sync_info = mybir.SyncInfo(on_wait=keep, on_update=list(si.on_update))
                    n_split += 1
                out.append(inst)
            if n_split:
                bb.instructions = out
    return n_split


def _emit(tc, t):
    nc = tc.nc
    from contextlib import ExitStack
    ctx = ExitStack()

    persist = ctx.enter_context(tc.tile_pool(name="persist", bufs=1))
    p_xrv = ctx.enter_context(tc.tile_pool(name="xrv", bufs=2))
    p_a = ctx.enter_context(tc.tile_pool(name="apool", bufs=8))
    p_dinvb = ctx.enter_context(tc.tile_pool(name="dinvb", bufs=3))
    p_small = ctx.enter_context(tc.tile_pool(name="small", bufs=2))
    p_fcr = ctx.enter_context(tc.tile_pool(name="fcr", bufs=8))
    p_out = ctx.enter_context(tc.tile_pool(name="outsb", bufs=4))
    p_s = ctx.enter_context(tc.tile_pool(name="pss", bufs=5, space="PSUM"))
    p_pv = ctx.enter_context(tc.tile_pool(name="pspv", bufs=3, space="PSUM"))
    p_pj = p_s

    # persistent buffers
    wq_r = persist.tile([128, DC, C], FP16, tag="wq")
    wk_r = persist.tile([128, DC, C], FP16, tag="wk")
    wv_r = persist.tile([128, DC, C], FP16, tag="wv")
    xq_r = persist.tile([128, DC, S], FP16, tag="xq")
    xk_r = persist.tile([128, DC, SK], FP16, tag="xk")
    qhT = persist.tile([128, NPAIR, S], FP16, tag="qhT")
    khT = persist.tile([128, NPAIR, SK], FP16, tag="khT")
    vhc = persist.tile([128, KC, HPC, DEPTH + 1], FP16, tag="vhc")
    ctxT = persist.tile([128, NPAIR, S], FP16, tag="ctxT")
    maskb = persist.tile([128, KC], F32, tag="maskb")
    ones1 = persist.tile([128, 1], F32, tag="ones1")

    # internal DRAM for the denominator shuttle: row r=pair*32+hh*16+qq of 128
    d_dram = nc.dram_tensor("d_dram", (NPAIR * 32, 128), F32, kind="Internal").ap()
    dinv_dram = nc.dram_tensor("dinv_dram", (NPAIR * 32, 128), F32, kind="Internal").ap()
    dinv_flat = dinv_dram.rearrange("a b -> (a b)")
    # [pair, hh, qw, 1024] view for the per-(pair,qb) D-row writes
    d_view4 = d_dram.rearrange("(pr h q j) f -> pr h q (j f)", h=2, q=NQW, j=QW // 128)

    nc.sync.dma_start(maskb[:], t["maskb"])
    nc.vector.memset(ones1[:], 1.0)
    nc.vector.tensor_copy(
        vhc[:, :, :, DEPTH:DEPTH + 1],
        ones1[:].to_broadcast([128, KC, HPC, 1]),
    )

    # ---- weight + input loads (already fp16 from the host), spread
    # across four engines' DMA queues so the startup isn't issue-bound ----
    engs = [nc.sync, nc.scalar, nc.gpsimd]
    ei = 0

    def load(dst_ap, src_ap):
        nonlocal ei
        engs[ei % 3].dma_start(dst_ap, src_ap)
        ei += 1

    wk_v = t["wkT"].rearrange("(dc p) c -> p dc c", p=128)
    wq_v = t["wqT"].rearrange("(dc p) c -> p dc c", p=128)
    wv_v = t["wvT"].rearrange("(dc p) c -> p dc c", p=128)
    xk_v = t["kcT"].rearrange("(dc p) s -> p dc s", p=128)
    xq_v = t["qT"].rearrange("(dc p) s -> p dc s", p=128)
    # criticality order: wk + xk feed the first projection, then wq + xq,
    # wv last (v-projection runs after proj_qk(0))
    load(wk_r[:], wk_v[:])
    for dc in range(DC):
        load(xk_r[:, dc, :], xk_v[:, dc, :])
    load(wq_r[:], wq_v[:])
    for dc in range(DC):
        load(xq_r[:, dc, :], xq_v[:, dc, :])
    load(wv_r[:], wv_v[:])

    def proj_qk_tasks(pair):
        """Closures, each projecting one 512-token block of c-tile `pair`."""
        tasks = []
        for x_r, w_r, dst, slen in ((xk_r, wk_r, khT, SK), (xq_r, wq_r, qhT, S)):
            for tb0 in range(0, slen, 512):
                tlen = min(512, slen - tb0)

                def task(x_r=x_r, w_r=w_r, dst=dst, tb0=tb0, tlen=tlen, pair=pair):
                    # pair 0 projects before attention starts: the score pool
                    # is idle then, so rotate through its 4 banks instead of
                    # serializing on the single filler bank
                    pool, tag = p_s, "s"
                    ps = pool.tile([128, 512], F32, tag=tag,
                                   name=f"pj_{pair}_{dst.name}_{tb0}")
                    for dc in range(DC):
                        nc.tensor.matmul(ps[:, :tlen],
                                         w_r[:, dc, pair * 128:(pair + 1) * 128],
                                         x_r[:, dc, tb0:tb0 + tlen],
                                         start=(dc == 0), stop=(dc == DC - 1))
                    nc.vector.tensor_copy(dst[:, pair, tb0:tb0 + tlen], ps[:, :tlen])
                tasks.append(task)
        return tasks

    def proj_v():
        vview = t["vcT"].rearrange("(dc p) s -> p dc s", p=128)
        for kt in range(KC):
            xrv = p_xrv.tile([128, DC, 128], FP16, tag="xrv", name=f"xrv_{kt}")
            nc.sync.dma_start(xrv[:], vview[:, :, kt * 128:(kt + 1) * 128])
            ps = p_s.tile([128, 512], F32, tag="s", name=f"psv_{kt}")
            for dc in range(DC):
                nc.tensor.matmul(ps[:, :C], xrv[:, dc, :], wv_r[:, dc, :],
                                 start=(dc == 0), stop=(dc == DC - 1))
            nc.vector.tensor_copy(
                vhc[:, kt, :, 0:DEPTH],
                ps[:, :C].rearrange("p (h d) -> p h d", h=HPC),
            )

    def attention(pair, fillers, qw_order=None):
        """fillers: {qw: closures} consumed evenly across that qw's steps."""
        steps = 2 * KC
        for qw in (qw_order or range(NQW)):
            filler = fillers.get(qw, [])
            n_fill = len(filler)
            step = 0
            dst_stage = p_dinvb.tile([64, QW], F32, tag="dinvb",
                                     name=f"dstage_{pair}_{qw}")
            for sh in range(2):
                q0 = qw * QW + sh * 512
                pv = [p_pv.tile([DEPTH + 1, 512], F32, tag="pv",
                                name=f"pv_{pair}_{qw}_{sh}_{hh}") for hh in range(2)]
                a_prev = None
                for kc in range(KC):
                    if (n_fill and
                            step * n_fill // steps != (step + 1) * n_fill // steps):
                        filler[step * n_fill // steps]()
                    step += 1
                    cur = []
                    for hh in range(2):
                        lo = 64 * hh
                        ps_s = p_s.tile([128, 512], F32, tag="s",
                                        name=f"s_{pair}_{qw}_{sh}_{kc}_{hh}")
                        nc.tensor.matmul(ps_s[:], khT[lo:lo + 64, pair, kc * 128:(kc + 1) * 128],
                                         qhT[lo:lo + 64, pair, q0:q0 + 512],
                                         start=True, stop=True)
                        a_t = p_a.tile([128, 512], FP16, tag="A",
                                       name=f"A_{pair}_{qw}_{sh}_{kc}_{hh}")
                        nc.scalar.activation(a_t[:], ps_s[:], EXP,
                                             bias=maskb[:, kc:kc + 1], scale=SCALE)
                        cur.append(a_t)
                    if kc >= 1:
                        for hh in range(2):
                            nc.tensor.matmul(pv[hh][:], vhc[:, kc - 1, 2 * pair + hh, :],
                                             a_prev[hh][:], start=(kc == 1), stop=False)
                    a_prev = cur
                for hh in range(2):
                    nc.tensor.matmul(pv[hh][:], vhc[:, KC - 1, 2 * pair + hh, :],
                                     a_prev[hh][:], start=False, stop=True)
                for hh in range(2):
                    nc.vector.tensor_copy(dst_stage[32 * hh:32 * hh + 1, sh * 512:(sh + 1) * 512],
                                          pv[hh][DEPTH:DEPTH + 1, :])
                    nc.vector.tensor_copy(ctxT[64 * hh:64 * hh + 64, pair, q0:q0 + 512],
                                          pv[hh][0:DEPTH, :])
            for hh in range(2):
                nc.gpsimd.dma_start(d_view4[pair, hh:hh + 1, qw, :],
                                    dst_stage[32 * hh:32 * hh + 1, :])
            # denominator pipeline for this (pair, qw)
            d128 = p_small.tile([16, 128], F32, tag="d128", name=f"d128_{pair}_{qw}")
            for hh in range(2):
                r0 = pair * 32 + hh * 16 + qw * 8
                nc.sync.dma_start(d128[8 * hh:8 * hh + 8, :], d_dram[r0:r0 + 8, :])
            dinv = p_small.tile([16, 128], F32, tag="dinv", name=f"dinv_{pair}_{qw}")
            nc.vector.reciprocal(dinv[:], d128[:])
            for hh in range(2):
                r0 = pair * 32 + hh * 16 + qw * 8
                nc.sync.dma_start(dinv_dram[r0:r0 + 8, :], dinv[8 * hh:8 * hh + 8, :])
            db = p_dinvb.tile([128, QW], F32, tag="dinvb", name=f"db_{pair}_{qw}")
            for cc in range(2):
                off = pair * 4096 + cc * 2048 + qw * QW
                nc.gpsimd.dma_start(db[64 * cc:64 * cc + 64, :],
                                    dinv_flat[off:off + QW].partition_broadcast(64))
            for hh in range(2):
                sl = ctxT[64 * hh:64 * hh + 64, pair, qw * QW:(qw + 1) * QW]
                nc.vector.tensor_mul(sl, sl, db[64 * hh:64 * hh + 64, :])

    # ---- fc task construction (emission deferred) ----
    fc_view = t["fcT"].rearrange("(pr p) e -> p pr e", p=128)
    o_view = t["o"].rearrange("(tt p) e -> p tt e", p=128)
    fcrs = []
    for ec in range(2):
        for pair in range(NPAIR):
            fcr = p_fcr.tile([128, 512], FP16, tag="fcr", name=f"fcr_{ec}_{pair}")
            nc.sync.dma_start(fcr[:], fc_view[:, pair, ec * 512:(ec + 1) * 512])
            fcrs.append(fcr)

    def fc_task(tt, ec, tail=False):
        def task():
            # tail groups run after attention: rotate through the then-idle
            # 4-slot score pool and copy via the idle ACT engine, so the
            # matmul groups stream instead of serializing on one bank
            pool, tag = p_s, "s"
            ps = pool.tile([128, 512], F32, tag=tag, name=f"fcps_{tt}_{ec}")
            for pair in range(NPAIR):
                nc.tensor.matmul(ps[:], ctxT[:, pair, tt * 128:(tt + 1) * 128],
                                 fcrs[ec * NPAIR + pair][:, :],
                                 start=(pair == 0), stop=(pair == NPAIR - 1))
            ob = p_out.tile([128, 512], F32, tag="outsb", name=f"ob_{tt}_{ec}")
            if tail:
                nc.scalar.copy(ob[:], ps[:])
            else:
                nc.vector.tensor_copy(ob[:], ps[:])
            nc.sync.dma_start(o_view[:, tt, ec * 512:(ec + 1) * 512], ob[:])
        return task

    # attention(3) runs qw=1 first; the fc groups that depend on qw=1
    # (tt 8..15) then fill its qw=0 half, and tt 0..7 run at the end.
    fc_fill = [fc_task(tt, ec) for tt in range(8, S // 128) for ec in range(2)]
    fc_tasks = [fc_task(tt, ec, tail=True) for tt in range(8) for ec in range(2)]

    # ---- schedule: v-proj and pair-0 k/q proj up front, then per-pair
    # attention with the next pair's projections interleaved ----
    for task in proj_qk_tasks(0):
        task()
    proj_v()
    for pair in range(NPAIR):
        if pair + 1 < NPAIR:
            nxt = proj_qk_tasks(pair + 1)
            attention(pair, {0: nxt[:4], 1: nxt[4:]})
        else:
            attention(pair, {0: fc_fill}, qw_order=[1, 0])

    # ---- fc projection: 32 closures; first half fills attention(3) ----
    for task in fc_tasks:
        task()

    ctx.close()


_NC_CACHE = {}


def _get_nc():
    if "nc" in _NC_CACHE:
        return _NC_CACHE["nc"]
    nc = bass.Bass("TRN2", target_bir_lowering=False, debug=False)
    t = {
        "qT": nc.dram_tensor("qT", (DM, S), FP16, kind="ExternalInput").ap(),
        "kcT": nc.dram_tensor("kcT", (DM, SK), FP16, kind="ExternalInput").ap(),
        "vcT": nc.dram_tensor("vcT", (DM, SK), FP16, kind="ExternalInput").ap(),
        "wqT": nc.dram_tensor("wqT", (DM, C), FP16, kind="ExternalInput").ap(),
        "wkT": nc.dram_tensor("wkT", (DM, C), FP16, kind="ExternalInput").ap(),
        "wvT": nc.dram_tensor("wvT", (DM, C), FP16, kind="ExternalInput").ap(),
        "fcT": nc.dram_tensor("fcT", (C, DM), FP16, kind="ExternalInput").ap(),
        "maskb": nc.dram_tensor("maskb", (128, KC), F32, kind="ExternalInput").ap(),
        "o": nc.dram_tensor("o", (S, DM), F32, kind="ExternalOutput").ap(),
    }
    with tile.TileContext(nc) as tc:
        _emit(tc, t)
    _split_excess_waits(nc)
    _NC_CACHE["nc"] = nc
    return nc


def _in_map_for_core(core, v, k, q, mask, wq, wk, wv, fc):
    b = core // 2
    hs = (core % 2) * HPC
    sel = np.nonzero(mask[b] == 0)[0]
    n = len(sel)
    assert n <= SK, f"unmasked key count {n} exceeds static SK={SK}"
    kc_ = np.zeros((SK, DM), np.float16)
    kc_[:n] = k[b][sel]
    vc_ = np.zeros((SK, DM), np.float16)
    vc_[:n] = v[b][sel]
    mb = np.full(SK, MASK_BIAS, np.float32)
    mb[:n] = 0.0
    f16 = np.float16
    return {
        "qT": np.ascontiguousarray(q[b].T.astype(f16)),
        "kcT": np.ascontiguousarray(kc_.T),
        "vcT": np.ascontiguousarray(vc_.T),
        "wqT": np.ascontiguousarray(wq[hs * DEPTH:(hs + HPC) * DEPTH].T.astype(f16)),
        "wkT": np.ascontiguousarray(wk[hs * DEPTH:(hs + HPC) * DEPTH].T.astype(f16)),
        "wvT": np.ascontiguousarray(wv[hs * DEPTH:(hs + HPC) * DEPTH].T.astype(f16)),
        "fcT": np.ascontiguousarray(fc[:, hs * DEPTH:(hs + HPC) * DEPTH].T.astype(f16)),
        "maskb": np.ascontiguousarray(mb.reshape(KC, 128).T),
    }


def kernel(v, k, q, mask, wq, wk, wv, fc, _run_kwargs=None):
    v = np.asarray(v, np.float32)
    k = np.asarray(k, np.float32)
    q = np.asarray(q, np.float32)
    mask = np.asarray(mask)
    wq = np.asarray(wq, np.float32)
    wk = np.asarray(wk, np.float32)
    wv = np.asarray(wv, np.float32)
    fc = np.asarray(fc, np.float32)

    nc = _get_nc()
    in_maps = [_in_map_for_core(c, v, k, q, mask, wq, wk, wv, fc)
               for c in range(NCORES)]
    res = run_bass_kernel_spmd(nc, in_maps, core_ids=list(range(NCORES)),
                               **(_run_kwargs or {}))
    outs = [r["o"] for r in res.results]
    full = np.stack([outs[2 * b] + outs[2 * b + 1] for b in range(B)])
    if _run_kwargs:
        kernel.last_results = res
    return full
